# revision 32
# baseline (speedup 1.0000x reference)
"""Single-head causal attention on 8 TRN2 NeuronCores, data-parallel over batch.

Reference computation (per batch b):
    Q = x[b] @ Wq; K = x[b] @ Wk; V = x[b] @ Wv          # [T, E]
    S = (Q @ K.T) / sqrt(E), causal-masked               # [T, T]
    P = softmax(S, axis=-1)
    out[b] = P @ V                                       # [T, E]

Shapes: B=8, T=2048, D=1024, E=128. One batch element per NeuronCore.

Device kernel strategy (S^T orientation — no on-device P transposes):
  - host feeds x[b].T as bf16 [D, T]; 1/sqrt(E) is folded into Wq.
  - Q^T, K^T, V^T = W.T @ x.T computed weight-stationary ([E, T] in SBUF).
  - V (natural [T, E]) obtained from V^T via DMA xbar transposes.
  - For each 512-wide q block: S^T chunks [k=128, q=512] = K^T_chunk.T @ Q^T,
    causal mask applied by adding -100 to masked entries (exp -> ~0),
    exp on ScalarE (no max subtraction needed: |S| <= ~6), giving P^T bf16.
    P@V accumulated in PSUM as O^T[e, q] = sum_k V_chunk.T @ P^T_chunk, and
    softmax denominators as rowsum[1, q] = sum_k ones.T @ P^T_chunk.
  - Output is unnormalized O^T [E, T] + rowsum [1, T]; host divides and
    transposes (exactly softmax, since exp(s)/sum(exp(s)) needs no max shift).
"""

import math
from contextlib import ExitStack

import numpy as np
import ml_dtypes

import concourse.bass as bass
import concourse.tile as tile
from concourse import bacc, mybir
from concourse._compat import with_exitstack
from concourse.bass_utils import run_bass_kernel_spmd

B, T, D, E = 8, 2048, 1024, 128
DC = D // 128  # contraction chunks for the projections
QB = 512       # q-block width (PSUM bank = 512 fp32)
NQB = T // QB  # 4 q blocks
NKT = T // 128 # 16 k chunks
MASK_NEG = -100.0

bf16 = mybir.dt.bfloat16
f32 = mybir.dt.float32


@with_exitstack
def _attention_body(ctx: ExitStack, tc: "tile.TileContext", rep: int,
                    xT, wq, wk, wv, outT, rowsum):
    nc = tc.nc
    singles = ctx.enter_context(tc.tile_pool(name=f"singles{rep}", bufs=1))
    pj_psum = ctx.enter_context(tc.tile_pool(name=f"pj_psum{rep}", bufs=2, space="PSUM"))
    st_psum = ctx.enter_context(tc.tile_pool(name=f"st_psum{rep}", bufs=3, space="PSUM"))
    ot_psum = ctx.enter_context(tc.tile_pool(name=f"ot_psum{rep}", bufs=2, space="PSUM"))
    rs_psum = ctx.enter_context(tc.tile_pool(name=f"rs_psum{rep}", bufs=1, space="PSUM"))
    pt_pool = ctx.enter_context(tc.tile_pool(name=f"pt{rep}", bufs=32))
    evac = ctx.enter_context(tc.tile_pool(name=f"evac{rep}", bufs=2))

    # --- inputs -> SBUF ---
    # weights arrive host-pre-chunked as [128, DC*E]: row p holds W[dc*128+p, e]
    # for dc-major, e-minor — so each partition's line is contiguous in DRAM.
    # Wk is loaded before x (first projection is K); Wv/Wq after x half 0.
    def load_w(name, w):
        wt = singles.tile([128, DC, E], bf16, tag=f"w_{name}")
        nc.sync.dma_start(wt[:], w.rearrange("p (dc e) -> p dc e", e=E))
        return wt
    # x loaded in [128, 1024] t-halves: fine enough that block-0/1 projections
    # start after ~2MB, coarse enough that the ~650ns/DMA HWDGE issue rate
    # doesn't throttle bandwidth. Half 1 is emitted later (see below) so the
    # HWDGE queue isn't hogged ahead of the V transposes.
    x_tiles = {}

    def load_x_half(h):
        for d in range(DC):
            xt = singles.tile([128, 2 * QB], bf16, tag=f"x_{d}_{h}")
            nc.sync.dma_start(
                xt[:], xT[d * 128:(d + 1) * 128, h * 2 * QB:(h + 1) * 2 * QB])
            x_tiles[(d, h)] = xt

    wk_t = load_w("wk", wk)
    wq_t = load_w("wq", wq)
    load_x_half(0)
    wv_t = load_w("wv", wv)

    # --- constants ---
    # tril mask [128, 128]: 0 where qf >= kp (keep), MASK_NEG where qf < kp
    tril = singles.tile([128, 128], f32, tag="tril")
    nc.gpsimd.memset(tril[:], 0.0)
    nc.gpsimd.affine_select(
        out=tril[:], in_=tril[:], compare_op=mybir.AluOpType.is_ge,
        fill=MASK_NEG, base=0, pattern=[[1, 128]], channel_multiplier=-1,
    )
    ones_t = singles.tile([128, 1], bf16, tag="ones")
    nc.gpsimd.memset(ones_t[:], 1.0)
    rs_sb = singles.tile([1, T], f32, tag="rs_sb")
    # warm up the ScalarE exp LUT so the table load is off the critical path
    warm = singles.tile([1, 1], f32, tag="warm")
    nc.gpsimd.memset(warm[:], 0.0)
    nc.scalar.activation(warm[:], warm[:], mybir.ActivationFunctionType.Exp)

    kT = singles.tile([128, T], bf16, tag="kT")
    vT = singles.tile([128, T], bf16, tag="vT")
    qT = singles.tile([128, T], bf16, tag="qT")
    v_nat = singles.tile([128, NKT, E], bf16, tag="v_nat")

    def project(wt, dst, tb):
        ps = pj_psum.tile([128, QB], f32, tag="pj")
        for d in range(DC):
            xt = x_tiles[(d, tb // 2)]
            col = (tb % 2) * QB
            nc.tensor.matmul(
                ps[:], lhsT=wt[:, d, :], rhs=xt[:, col:col + QB],
                start=(d == 0), stop=(d == DC - 1),
            )
        nc.vector.tensor_copy(dst[:, tb * QB:(tb + 1) * QB], ps[:])

    def qlo(kt, qb):  # first valid in-block q column for this k chunk
        m = kt - 4 * qb
        return 128 * m if m > 0 else 0

    def block_kts(qb):
        return list(range(min(NKT - 1, 4 * qb + 3) + 1))

    pt_tiles = {}  # (qb, kt) -> SBUF tile holding exp(S^T) bf16

    def s_exp_block(qb):
        # S^T chunks + causal mask + exp, plus the rowsum path (quad-sums of
        # the exp'd chunks + one ones-matmul per quad). Emitted as early as
        # its inputs (kT chunks <= last kt, qT block qb) allow, since the exp
        # chain on ScalarE is the kernel's scarcest resource; the rowsum path
        # is the longest dependency tail, so it starts here, not in pv_block.
        kts = block_kts(qb)
        for kt in kts:
            lo = qlo(kt, qb)
            st = st_psum.tile([128, QB], f32, tag="st")
            nc.tensor.matmul(
                st[:, lo:QB], lhsT=kT[:, kt * 128:(kt + 1) * 128],
                rhs=qT[:, qb * QB + lo:(qb + 1) * QB], start=True, stop=True,
            )
            if kt >= 4 * qb:  # diagonal chunk: mask the leading 128-col triangle
                nc.vector.tensor_add(st[:, lo:lo + 128], st[:, lo:lo + 128], tril[:])
            pt = pt_pool.tile([128, QB], bf16, tag="pt")
            nc.scalar.activation(pt[:, lo:QB], st[:, lo:QB],
                                 mybir.ActivationFunctionType.Exp)
            pt_tiles[(qb, kt)] = pt
        # rowsums: combine each quad of exp'd chunks into a fresh tile with 3
        # bf16 DVE adds (exact valid ranges, so no garbage enters), then one
        # ones-matmul per quad — 4x fewer TensorE rs streams. Fresh tiles (not
        # in-place) keep the pt chunks intact for pv_block's reads.
        rs = rs_psum.tile([1, QB], f32, tag="rs")
        quads = [kts[g * 4:(g + 1) * 4] for g in range((len(kts) + 3) // 4)]
        for g, quad in enumerate(quads):
            q0, q1, q2, q3 = quad
            los = [qlo(kt, qb) for kt in quad]
            qsum = pt_pool.tile([128, QB], bf16, tag="qsum")
            nc.vector.tensor_add(
                qsum[:, los[1]:QB], pt_tiles[(qb, q0)][:, los[1]:QB],
                pt_tiles[(qb, q1)][:, los[1]:QB])
            if los[1] > 0:  # diagonal quad: q0's leading columns missed above
                nc.vector.tensor_copy(
                    qsum[:, 0:los[1]], pt_tiles[(qb, q0)][:, 0:los[1]])
            nc.vector.tensor_add(
                qsum[:, los[2]:QB], qsum[:, los[2]:QB],
                pt_tiles[(qb, q2)][:, los[2]:QB])
            nc.vector.tensor_add(
                qsum[:, los[3]:QB], qsum[:, los[3]:QB],
                pt_tiles[(qb, q3)][:, los[3]:QB])
            nc.tensor.matmul(
                rs[:], lhsT=ones_t[:], rhs=qsum[:],
                start=(g == 0), stop=(g == len(quads) - 1),
            )
        nc.vector.tensor_copy(rs_sb[:, qb * QB:(qb + 1) * QB], rs[:])

    def pv_block(qb):
        # PV accumulation + evacuation. kt ascends: the first (start=True)
        # matmul of the PSUM accumulation group is full-width, so later
        # narrower diagonal-chunk matmuls only touch already-initialized
        # bytes (PSUM zero_out is per-matmul).
        kts = block_kts(qb)
        ot = ot_psum.tile([128, QB], f32, tag="ot")
        for i, kt in enumerate(kts):
            lo = qlo(kt, qb)
            nc.tensor.matmul(
                ot[:, lo:QB], lhsT=v_nat[:, kt, :], rhs=pt_tiles[(qb, kt)][:, lo:QB],
                start=(i == 0), stop=(i == len(kts) - 1),
            )
        oe = evac.tile([128, QB], f32, tag="oe")
        nc.vector.tensor_copy(oe[:], ot[:])
        nc.sync.dma_start(outT[:, qb * QB:(qb + 1) * QB], oe[:])

    # The attention phase is ScalarE(exp)-bound, so the schedule is built to
    # start the LONGEST exp chain (qb=3, 16 chunks) as early as possible:
    # project all of K plus Q block 3, emit attention(3) — its S matmuls and
    # exps begin while TensorE continues with the V projections (whose
    # results PV(3) needs anyway) — then descend through the remaining blocks
    # so the shortest exp chain (qb=0) sits in the exposed tail.
    project(wk_t, kT, 0)
    project(wk_t, kT, 1)
    load_x_half(1)
    project(wk_t, kT, 2)
    project(wk_t, kT, 3)
    project(wq_t, qT, 3)
    s_exp_block(3)
    for tb in range(NQB):
        project(wv_t, vT, tb)
        # V natural [t, e] chunks 4tb..4tb+3, stored [128 t_in, kt, e]
        # (xbar transpose semantics verified: out[p, c, e] = in.T[c*128+p, e])
        nc.sync.dma_start_transpose(
            v_nat[:, 4 * tb:4 * (tb + 1), :], vT[:, tb * QB:(tb + 1) * QB])
    project(wq_t, qT, 2)
    s_exp_block(2)
    pv_block(3)
    project(wq_t, qT, 1)
    s_exp_block(1)
    pv_block(2)
    project(wq_t, qT, 0)
    s_exp_block(0)
    pv_block(1)
    pv_block(0)
    nc.sync.dma_start(rowsum[:], rs_sb[:])


def build(reps: int = 1) -> "bacc.Bacc":
    nc = bacc.Bacc("TRN2", target_bir_lowering=False, debug=False,
                   enable_asserts=False, num_devices=B)
    xT = nc.dram_tensor("xT", [D, T], bf16, kind="ExternalInput").ap()
    wq = nc.dram_tensor("Wq", [128, DC * E], bf16, kind="ExternalInput").ap()
    wk = nc.dram_tensor("Wk", [128, DC * E], bf16, kind="ExternalInput").ap()
    wv = nc.dram_tensor("Wv", [128, DC * E], bf16, kind="ExternalInput").ap()
    outT = nc.dram_tensor("outT", [E, T], f32, kind="ExternalOutput").ap()
    rowsum = nc.dram_tensor("rowsum", [1, T], f32, kind="ExternalOutput").ap()
    with tile.TileContext(nc) as tc:
        for rep in range(reps):
            _attention_body(tc, rep, xT, wq, wk, wv, outT, rowsum)
    nc.compile()
    return nc


def _chunk_w(w):
    # [D, E] -> [128, DC*E] with row p = concat over dc of W[dc*128+p, :]
    return np.ascontiguousarray(
        np.asarray(w).reshape(DC, 128, E).transpose(1, 0, 2).reshape(128, DC * E)
    )


def make_in_maps(x, Wq, Wk, Wv):
    scale = 1.0 / math.sqrt(E)
    xT = np.ascontiguousarray(x.transpose(0, 2, 1)).astype(ml_dtypes.bfloat16)
    wq = _chunk_w(np.asarray(Wq) * scale).astype(ml_dtypes.bfloat16)
    wk = _chunk_w(Wk).astype(ml_dtypes.bfloat16)
    wv = _chunk_w(Wv).astype(ml_dtypes.bfloat16)
    return [{"xT": xT[b], "Wq": wq, "Wk": wk, "Wv": wv} for b in range(B)]


def postprocess(results):
    out = np.empty((B, T, E), dtype=np.float32)
    for b in range(B):
        oT = np.asarray(results[b]["outT"])          # [E, T] unnormalized
        rs = np.asarray(results[b]["rowsum"])[0]     # [T]
        out[b] = (oT / rs[None, :]).T
    return out


_NC_CACHE = {}


def kernel(x, Wq, Wk, Wv):
    x = np.asarray(x)
    if 1 not in _NC_CACHE:
        _NC_CACHE[1] = build(reps=1)
    nc = _NC_CACHE[1]
    in_maps = make_in_maps(x, Wq, Wk, Wv)
    res = run_bass_kernel_spmd(nc, in_maps, core_ids=list(range(B)))
    return postprocess(res.results)


if __name__ == "__main__":
    rng = np.random.default_rng(0)
    x = rng.standard_normal((B, T, D), dtype=np.float32)
    Wq = rng.standard_normal((D, E), dtype=np.float32) / math.sqrt(D)
    Wk = rng.standard_normal((D, E), dtype=np.float32) / math.sqrt(D)
    Wv = rng.standard_normal((D, E), dtype=np.float32) / math.sqrt(D)
    out = kernel(x, Wq, Wk, Wv)
    print("out", out.shape, out.dtype, np.abs(out).max())


# revision 41
# speedup vs baseline: 637.9452x; 637.9452x over previous
"""Single-head causal attention on 8 TRN2 NeuronCores, data-parallel over batch.

Reference computation (per batch b):
    Q = x[b] @ Wq; K = x[b] @ Wk; V = x[b] @ Wv          # [T, E]
    S = (Q @ K.T) / sqrt(E), causal-masked               # [T, T]
    P = softmax(S, axis=-1)
    out[b] = P @ V                                       # [T, E]

Shapes: B=8, T=2048, D=1024, E=128. One batch element per NeuronCore.

Device kernel strategy (S^T orientation — no on-device P transposes):
  - host feeds x[b].T as bf16 [D, T]; 1/sqrt(E) is folded into Wq.
  - Q^T, K^T, V^T = W.T @ x.T computed weight-stationary ([E, T] in SBUF).
  - V (natural [T, E]) obtained from V^T via DMA xbar transposes.
  - For each 512-wide q block: S^T chunks [k=128, q=512] = K^T_chunk.T @ Q^T,
    causal mask applied by adding -100 to masked entries (exp -> ~0),
    exp on ScalarE (no max subtraction needed: |S| <= ~6), giving P^T bf16.
    P@V accumulated in PSUM as O^T[e, q] = sum_k V_chunk.T @ P^T_chunk, and
    softmax denominators as rowsum[1, q] = sum_k ones.T @ P^T_chunk.
  - Output is unnormalized O^T [E, T] + rowsum [1, T]; host divides and
    transposes (exactly softmax, since exp(s)/sum(exp(s)) needs no max shift).
"""

import math
from contextlib import ExitStack

import numpy as np
import ml_dtypes

import concourse.bass as bass
import concourse.tile as tile
from concourse import bacc, mybir
from concourse._compat import with_exitstack
from concourse.bass_utils import run_bass_kernel_spmd

B, T, D, E = 8, 2048, 1024, 128
DC = D // 128  # contraction chunks for the projections
QB = 512       # q-block width (PSUM bank = 512 fp32)
NQB = T // QB  # 4 q blocks
NKT = T // 128 # 16 k chunks
MASK_NEG = -100.0

bf16 = mybir.dt.bfloat16
f32 = mybir.dt.float32


@with_exitstack
def _attention_body(ctx: ExitStack, tc: "tile.TileContext", rep: int,
                    xT, wq, wk, wv, outT, rowsum):
    nc = tc.nc
    singles = ctx.enter_context(tc.tile_pool(name=f"singles{rep}", bufs=1))
    pj_psum = ctx.enter_context(tc.tile_pool(name=f"pj_psum{rep}", bufs=2, space="PSUM"))
    st_psum = ctx.enter_context(tc.tile_pool(name=f"st_psum{rep}", bufs=3, space="PSUM"))
    ot_psum = ctx.enter_context(tc.tile_pool(name=f"ot_psum{rep}", bufs=2, space="PSUM"))
    rs_psum = ctx.enter_context(tc.tile_pool(name=f"rs_psum{rep}", bufs=1, space="PSUM"))
    pt_pool = ctx.enter_context(tc.tile_pool(name=f"pt{rep}", bufs=32))
    evac = ctx.enter_context(tc.tile_pool(name=f"evac{rep}", bufs=2))

    # --- inputs -> SBUF ---
    # weights arrive host-pre-chunked as [128, DC*E]: row p holds W[dc*128+p, e]
    # for dc-major, e-minor — so each partition's line is contiguous in DRAM.
    # Wk is loaded before x (first projection is K); Wv/Wq after x half 0.
    def load_w(name, w):
        wt = singles.tile([128, DC, E], bf16, tag=f"w_{name}")
        nc.sync.dma_start(wt[:], w.rearrange("p (dc e) -> p dc e", e=E))
        return wt
    # x loaded in [128, 1024] t-halves: fine enough that block-0/1 projections
    # start after ~2MB, coarse enough that the ~650ns/DMA HWDGE issue rate
    # doesn't throttle bandwidth. Half 1 is emitted later (see below) so the
    # HWDGE queue isn't hogged ahead of the V transposes.
    x_tiles = {}

    def load_x_half(h):
        for d in range(DC):
            xt = singles.tile([128, 2 * QB], bf16, tag=f"x_{d}_{h}")
            nc.sync.dma_start(
                xt[:], xT[d * 128:(d + 1) * 128, h * 2 * QB:(h + 1) * 2 * QB])
            x_tiles[(d, h)] = xt

    wk_t = load_w("wk", wk)
    load_x_half(0)
    wq_t = load_w("wq", wq)
    wv_t = load_w("wv", wv)

    # --- constants ---
    # tril mask [128, 128]: 0 where qf >= kp (keep), MASK_NEG where qf < kp
    tril = singles.tile([128, 128], f32, tag="tril")
    nc.gpsimd.memset(tril[:], 0.0)
    nc.gpsimd.affine_select(
        out=tril[:], in_=tril[:], compare_op=mybir.AluOpType.is_ge,
        fill=MASK_NEG, base=0, pattern=[[1, 128]], channel_multiplier=-1,
    )
    ones_t = singles.tile([128, 1], bf16, tag="ones")
    nc.gpsimd.memset(ones_t[:], 1.0)
    rs_sb = singles.tile([1, T], f32, tag="rs_sb")
    # warm up the ScalarE exp LUT so the table load is off the critical path
    warm = singles.tile([1, 1], f32, tag="warm")
    nc.gpsimd.memset(warm[:], 0.0)
    nc.scalar.activation(warm[:], warm[:], mybir.ActivationFunctionType.Exp)

    kT = singles.tile([128, T], bf16, tag="kT")
    vT = singles.tile([128, T], bf16, tag="vT")
    qT = singles.tile([128, T], bf16, tag="qT")
    v_nat = singles.tile([128, NKT, E], bf16, tag="v_nat")

    def project(wt, dst, tb):
        ps = pj_psum.tile([128, QB], f32, tag="pj")
        for d in range(DC):
            xt = x_tiles[(d, tb // 2)]
            col = (tb % 2) * QB
            nc.tensor.matmul(
                ps[:], lhsT=wt[:, d, :], rhs=xt[:, col:col + QB],
                start=(d == 0), stop=(d == DC - 1),
            )
        nc.vector.tensor_copy(dst[:, tb * QB:(tb + 1) * QB], ps[:])

    def qlo(kt, qb):  # first valid in-block q column for this k chunk
        m = kt - 4 * qb
        return 128 * m if m > 0 else 0

    def block_kts(qb):
        return list(range(min(NKT - 1, 4 * qb + 3) + 1))

    pt_tiles = {}  # (qb, kt) -> SBUF tile holding exp(S^T) bf16

    def s_exp_block(qb):
        # S^T chunks + causal mask + exp, plus the rowsum path (quad-sums of
        # the exp'd chunks + one ones-matmul per quad). Emitted as early as
        # its inputs (kT chunks <= last kt, qT block qb) allow, since the exp
        # chain on ScalarE is the kernel's scarcest resource; the rowsum path
        # is the longest dependency tail, so it starts here, not in pv_block.
        kts = block_kts(qb)
        for kt in kts:
            lo = qlo(kt, qb)
            st = st_psum.tile([128, QB], f32, tag="st")
            nc.tensor.matmul(
                st[:, lo:QB], lhsT=kT[:, kt * 128:(kt + 1) * 128],
                rhs=qT[:, qb * QB + lo:(qb + 1) * QB], start=True, stop=True,
            )
            if kt >= 4 * qb:  # diagonal chunk: mask the leading 128-col triangle
                nc.vector.tensor_add(st[:, lo:lo + 128], st[:, lo:lo + 128], tril[:])
            pt = pt_pool.tile([128, QB], bf16, tag="pt")
            nc.scalar.activation(pt[:, lo:QB], st[:, lo:QB],
                                 mybir.ActivationFunctionType.Exp)
            pt_tiles[(qb, kt)] = pt
        # rowsums: combine each quad of exp'd chunks into a fresh tile with 3
        # bf16 DVE adds (exact valid ranges, so no garbage enters), then one
        # ones-matmul per quad — 4x fewer TensorE rs streams. Fresh tiles (not
        # in-place) keep the pt chunks intact for pv_block's reads.
        rs = rs_psum.tile([1, QB], f32, tag="rs")
        quads = [kts[g * 4:(g + 1) * 4] for g in range((len(kts) + 3) // 4)]
        for g, quad in enumerate(quads):
            q0, q1, q2, q3 = quad
            los = [qlo(kt, qb) for kt in quad]
            qsum = pt_pool.tile([128, QB], bf16, tag="qsum")
            nc.vector.tensor_add(
                qsum[:, los[1]:QB], pt_tiles[(qb, q0)][:, los[1]:QB],
                pt_tiles[(qb, q1)][:, los[1]:QB])
            if los[1] > 0:  # diagonal quad: q0's leading columns missed above
                nc.vector.tensor_copy(
                    qsum[:, 0:los[1]], pt_tiles[(qb, q0)][:, 0:los[1]])
            nc.vector.tensor_add(
                qsum[:, los[2]:QB], qsum[:, los[2]:QB],
                pt_tiles[(qb, q2)][:, los[2]:QB])
            nc.vector.tensor_add(
                qsum[:, los[3]:QB], qsum[:, los[3]:QB],
                pt_tiles[(qb, q3)][:, los[3]:QB])
            nc.tensor.matmul(
                rs[:], lhsT=ones_t[:], rhs=qsum[:],
                start=(g == 0), stop=(g == len(quads) - 1),
            )
        nc.vector.tensor_copy(rs_sb[:, qb * QB:(qb + 1) * QB], rs[:])

    def pv_block(qb):
        # PV accumulation + evacuation. kt ascends: the first (start=True)
        # matmul of the PSUM accumulation group is full-width, so later
        # narrower diagonal-chunk matmuls only touch already-initialized
        # bytes (PSUM zero_out is per-matmul).
        kts = block_kts(qb)
        ot = ot_psum.tile([128, QB], f32, tag="ot")
        for i, kt in enumerate(kts):
            lo = qlo(kt, qb)
            nc.tensor.matmul(
                ot[:, lo:QB], lhsT=v_nat[:, kt, :], rhs=pt_tiles[(qb, kt)][:, lo:QB],
                start=(i == 0), stop=(i == len(kts) - 1),
            )
        oe = evac.tile([128, QB], f32, tag="oe")
        nc.vector.tensor_copy(oe[:], ot[:])
        nc.sync.dma_start(outT[:, qb * QB:(qb + 1) * QB], oe[:])

    # The attention phase is ScalarE(exp)-bound, so the schedule is built to
    # start the LONGEST exp chain (qb=3, 16 chunks) as early as possible:
    # project all of K plus Q block 3, emit attention(3) — its S matmuls and
    # exps begin while TensorE continues with the V projections (whose
    # results PV(3) needs anyway) — then descend through the remaining blocks
    # so the shortest exp chain (qb=0) sits in the exposed tail.
    project(wk_t, kT, 0)
    project(wk_t, kT, 1)
    load_x_half(1)
    project(wk_t, kT, 2)
    project(wk_t, kT, 3)
    project(wq_t, qT, 3)
    s_exp_block(3)
    for tb in range(NQB):
        project(wv_t, vT, tb)
        # V natural [t, e] chunks 4tb..4tb+3, stored [128 t_in, kt, e]
        # (xbar transpose semantics verified: out[p, c, e] = in.T[c*128+p, e])
        nc.sync.dma_start_transpose(
            v_nat[:, 4 * tb:4 * (tb + 1), :], vT[:, tb * QB:(tb + 1) * QB])
    project(wq_t, qT, 2)
    s_exp_block(2)
    pv_block(3)
    project(wq_t, qT, 1)
    s_exp_block(1)
    pv_block(2)
    project(wq_t, qT, 0)
    s_exp_block(0)
    pv_block(1)
    pv_block(0)
    nc.sync.dma_start(rowsum[:], rs_sb[:])


def build(reps: int = 1) -> "bacc.Bacc":
    nc = bacc.Bacc("TRN2", target_bir_lowering=False, debug=False,
                   enable_asserts=False, num_devices=B)
    xT = nc.dram_tensor("xT", [D, T], bf16, kind="ExternalInput").ap()
    wq = nc.dram_tensor("Wq", [128, DC * E], bf16, kind="ExternalInput").ap()
    wk = nc.dram_tensor("Wk", [128, DC * E], bf16, kind="ExternalInput").ap()
    wv = nc.dram_tensor("Wv", [128, DC * E], bf16, kind="ExternalInput").ap()
    outT = nc.dram_tensor("outT", [E, T], f32, kind="ExternalOutput").ap()
    rowsum = nc.dram_tensor("rowsum", [1, T], f32, kind="ExternalOutput").ap()
    with tile.TileContext(nc) as tc:
        for rep in range(reps):
            _attention_body(tc, rep, xT, wq, wk, wv, outT, rowsum)
    nc.compile()
    return nc


def _chunk_w(w):
    # [D, E] -> [128, DC*E] with row p = concat over dc of W[dc*128+p, :]
    return np.ascontiguousarray(
        np.asarray(w).reshape(DC, 128, E).transpose(1, 0, 2).reshape(128, DC * E)
    )


def make_in_maps(x, Wq, Wk, Wv):
    scale = 1.0 / math.sqrt(E)
    xT = np.ascontiguousarray(x.transpose(0, 2, 1)).astype(ml_dtypes.bfloat16)
    wq = _chunk_w(np.asarray(Wq) * scale).astype(ml_dtypes.bfloat16)
    wk = _chunk_w(Wk).astype(ml_dtypes.bfloat16)
    wv = _chunk_w(Wv).astype(ml_dtypes.bfloat16)
    return [{"xT": xT[b], "Wq": wq, "Wk": wk, "Wv": wv} for b in range(B)]


def postprocess(results):
    out = np.empty((B, T, E), dtype=np.float32)
    for b in range(B):
        oT = np.asarray(results[b]["outT"])          # [E, T] unnormalized
        rs = np.asarray(results[b]["rowsum"])[0]     # [T]
        out[b] = (oT / rs[None, :]).T
    return out


_NC_CACHE = {}


def kernel(x, Wq, Wk, Wv):
    x = np.asarray(x)
    if 1 not in _NC_CACHE:
        _NC_CACHE[1] = build(reps=1)
    nc = _NC_CACHE[1]
    in_maps = make_in_maps(x, Wq, Wk, Wv)
    res = run_bass_kernel_spmd(nc, in_maps, core_ids=list(range(B)))
    return postprocess(res.results)


if __name__ == "__main__":
    rng = np.random.default_rng(0)
    x = rng.standard_normal((B, T, D), dtype=np.float32)
    Wq = rng.standard_normal((D, E), dtype=np.float32) / math.sqrt(D)
    Wk = rng.standard_normal((D, E), dtype=np.float32) / math.sqrt(D)
    Wv = rng.standard_normal((D, E), dtype=np.float32) / math.sqrt(D)
    out = kernel(x, Wq, Wk, Wv)
    print("out", out.shape, out.dtype, np.abs(out).max())
